# revision 41
# baseline (speedup 1.0000x reference)
"""Trainium2 Bass kernel for a dense transformer block.

Model (B=4, N=1024, D=1024, H=16, hd=64):
  q/k/v = x{q,k,v} @ W{q,k,v}.T ; attn = softmax(mask(q k^T / 8)) @ v
  x1 = LN1(x_q + attn_out @ Wp.T + bp)
  out = LN2(x1 + relu(x1 @ W1.T + bf1) @ W2.T + bf2)

Sharding: 8 cores = (batch b, query-half qh). Each core owns 512 queries of
one batch; full K/V for that batch are recomputed per core (no collectives).

Device layout is feature-major: activations live as x^T[d, n] with
d = mt*128 + p (p = SBUF partition, mt = 8 free tiles). Reductions over d or
kv (softmax denom, LayerNorm) go through the PE via ones-vector matmuls / a
mask column appended to V.

Placement choices (v4):
 - All matmul operands bf16 (host-prepared); PSUM accumulates fp32.
 - Mask applied host-side: masked kv rows of x_v zeroed, mask vector becomes
   V's denominator column, so device softmax is a pure exp and one exp
   instruction spans two kv tiles ([128,1024] over a 2-bank PSUM tile).
 - V proj emitted first (its DMAs arrive first), then Q proj, then K proj
   interleaved per-m-tile with S/exp/O so the Act-engine exp chain overlaps
   PE work. Warm-up matmuls keep the PE p-state ramped during the DMA lead-in.
 - LN uses sqrt+reciprocal (not ln/exp) so the whole kernel needs one act
   table switch, hoisted off the critical path via a dummy sqrt.
 - LN1's affine is folded into W1/bf1 (host) + the FFN2 merge; LN2's affine
   is applied on the host. LN applies are two all-bf16 DVE ops per tile.
"""
import numpy as np

P = 128
DIM = 1024
HEADS = 16
HD = 64
B = 4
NQ = 1024
NKV = 1024
TQ = 512          # queries per core
MT = DIM // P     # 8 feature tiles
NJ = NKV // P     # 8 kv tiles
EPS = 1e-8
SCALE = HD ** -0.5

_CACHE = {}


def _build():
    import concourse.bass as bass
    import concourse.mybir as mybir
    import concourse.tile as tile
    from concourse import bacc

    f32 = mybir.dt.float32
    bf16 = mybir.dt.bfloat16
    AF = mybir.ActivationFunctionType
    OP = mybir.AluOpType

    nc = bacc.Bacc("TRN2", target_bir_lowering=False, debug=False)

    xqT_d = nc.dram_tensor("xqT", [P, MT, TQ], bf16, kind="ExternalInput").ap()
    xkT_d = nc.dram_tensor("xkT", [P, MT, NKV], bf16, kind="ExternalInput").ap()
    xvT_d = nc.dram_tensor("xvT", [P, MT, NKV], bf16, kind="ExternalInput").ap()
    wv_d = nc.dram_tensor("wv_r", [P, MT, DIM], bf16, kind="ExternalInput").ap()
    wq_d = nc.dram_tensor("wq", [MT, P, MT, P], bf16, kind="ExternalInput").ap()
    wt_d = {}
    for w in ("wk", "wp", "w1", "w2"):
        # [p_k, mt_m, mt_k, p_m]: whole weight, contiguous per partition
        wt_d[w] = nc.dram_tensor(w, [P, MT, MT, P], bf16, kind="ExternalInput").ap()
    mcol_d = nc.dram_tensor("mcol", [P, NJ], bf16, kind="ExternalInput").ap()
    vec_d = {}
    for v in ("bp", "bf1", "g1", "b1f2", "g2", "b2"):
        vec_d[v] = nc.dram_tensor(v, [P, MT], f32, kind="ExternalInput").ap()
    onesc_d = nc.dram_tensor("onesc", [P, 1], bf16, kind="ExternalInput").ap()
    onesr_d = nc.dram_tensor("onesr", [1, P], bf16, kind="ExternalInput").ap()
    out_d = nc.dram_tensor("out", [P, MT, TQ], bf16, kind="ExternalOutput").ap()

    with tile.TileContext(nc) as tc, \
         nc.allow_low_precision(reason="bf16 matmul pipeline, fp32 psum accum"):
        with tc.tile_pool(name="persist", bufs=1) as pp, \
             tc.tile_pool(name="wstream", bufs=3) as wp, \
             tc.tile_pool(name="wbig", bufs=3) as wbp, \
             tc.tile_pool(name="ptile", bufs=4) as ppool, \
             tc.tile_pool(name="small", bufs=6) as sp, \
             tc.tile_pool(name="sq", bufs=2) as sqp, \
             tc.tile_pool(name="tmerge", bufs=2) as tmp_pool, \
             tc.tile_pool(name="mmps", bufs=2, space="PSUM") as mmps, \
             tc.tile_pool(name="sps", bufs=2, space="PSUM") as sps, \
             tc.tile_pool(name="obps", bufs=2, space="PSUM") as obps:

            # ---- persistent tiles ----
            xqT = pp.tile([P, MT, TQ], bf16, tag="xqT")
            q_sb = pp.tile([P, MT, TQ], bf16, tag="q_sb")
            xkT = pp.tile([P, MT, NKV], bf16, tag="big_a", name="xkT")
            xvT = pp.tile([P, MT, NKV], bf16, tag="big_b", name="xvT")
            kT = pp.tile([P, MT, NKV], bf16, tag="kT")
            wv = pp.tile([P, MT, DIM], bf16, tag="wv")
            v_sb = pp.tile([P, NJ, HEADS, HD + 1], bf16, tag="v_sb")
            mcol = pp.tile([P, NJ], bf16, tag="mcol")
            onesc = pp.tile([P, 1], bf16, tag="onesc")
            onesr = pp.tile([1, P], bf16, tag="onesr")
            zt = pp.tile([P, TQ], bf16, tag="zt")
            vec = {v: pp.tile([P, MT], f32, tag=f"vec_{v}", name=f"sb_{v}")
                   for v in vec_d}

            # PE warm-up on zeros: keeps the p-state ramp hot through the
            # initial DMA wait
            nc.gpsimd.memset(zt[:], 0.0)
            eps_t = pp.tile([1, 1], f32, tag="eps")
            nc.gpsimd.memset(eps_t[:], EPS)
            wu_ps = mmps.tile([P, TQ], f32, tag="mm", name="warmup")
            for i in range(24):
                nc.tensor.matmul(wu_ps[:], zt[:, 0:P], zt[:],
                                 start=True, stop=True)

            # DMAs spread over SP/Act/DVE queues: a queue's DMAs serialize
            # through transfer completion, so each queue gets an independent
            # need-ordered chain and the shared DMA engines see transfers in
            # global need order.
            nc.sync.dma_start(wv[:, :, 0:512], wv_d[:, :, 0:512])
            nc.scalar.dma_start(xvT[:], xvT_d)
            nc.sync.dma_start(wv[:, :, 512:1024], wv_d[:, :, 512:1024])
            nc.scalar.dma_start(xqT[:], xqT_d)
            wq_t = []
            for mt in range(MT):
                t = wp.tile([P, MT, P], bf16, tag="w", name=f"wq{mt}")
                nc.sync.dma_start(t[:], wq_d[mt])
                wq_t.append(t)
            nc.scalar.dma_start(xkT[:], xkT_d)
            wbig = {}
            for w in ("wk", "wp", "w1", "w2"):
                wbig[w] = wbp.tile([P, MT, MT, P], bf16, tag="wb", name=w)
                nc.sync.dma_start(wbig[w][:], wt_d[w])
            # tiny DMAs after the big ones (Act queue)
            nc.scalar.dma_start(mcol[:], mcol_d)
            for v in vec_d:
                nc.scalar.dma_start(vec[v][:], vec_d[v])
            nc.scalar.dma_start(onesc[:], onesc_d)
            nc.scalar.dma_start(onesr[:], onesr_d)

            # ---- V projection (swapped roles: out partitions = kv) ----
            for j in range(NJ):
                for half in range(2):
                    ps = mmps.tile([P, TQ], f32, tag="mm")
                    for kt in range(MT):
                        nc.tensor.matmul(
                            ps[:], xvT[:, kt, j * P:(j + 1) * P],
                            wv[:, kt, half * 512:(half + 1) * 512],
                            start=(kt == 0), stop=(kt == MT - 1))
                    # Pool/GPSIMD cannot read PSUM on HW; DVE is idle here
                    nc.vector.tensor_copy(
                        v_sb[:, j, 8 * half:8 * half + 8, 0:HD],
                        ps[:].rearrange("p (h d) -> p h d", h=8))

            # V's denominator column: mask value per kv row, replicated per
            # head (on Pool, which has no other work until the FFN phase, so
            # waiting for mcol head-blocks nothing)
            for h in range(HEADS):
                nc.gpsimd.tensor_copy(v_sb[:, :, h, HD:HD + 1],
                                      mcol[:].unsqueeze(-1))

            # ---- Q projection (Act engine moves PSUM -> q_sb) ----
            for mt in range(MT):
                ps = mmps.tile([P, TQ], f32, tag="mm")
                for kt in range(MT):
                    nc.tensor.matmul(ps[:], wq_t[mt][:, kt, :], xqT[:, kt, :],
                                     start=(kt == 0), stop=(kt == MT - 1))
                nc.scalar.copy(q_sb[:, mt, :], ps[:])

            # ---- interleaved K projection + attention ----
            o_sb = pp.tile([P, MT, TQ], bf16, tag="o_sb")

            def k_proj_mt(mt):
                for half in range(2):
                    ps = mmps.tile([P, TQ], f32, tag="mm")
                    for kt in range(MT):
                        nc.tensor.matmul(
                            ps[:], wbig["wk"][:, mt, kt, :],
                            xkT[:, kt, half * 512:(half + 1) * 512],
                            start=(kt == 0), stop=(kt == MT - 1))
                    nc.vector.tensor_copy(
                        kT[:, mt, half * 512:(half + 1) * 512], ps[:])

            def attn_head(h):
                lo = 64 * (h % 2)
                mtl = h // 2
                p_tiles = []
                for pr in range(4):       # two kv tiles per psum tile / exp
                    s_ps = sps.tile([P, 2 * TQ], f32, tag="s",
                                    name=f"s{h}_{pr}")
                    for jj in range(2):
                        j = 2 * pr + jj
                        nc.tensor.matmul(
                            s_ps[:, jj * TQ:(jj + 1) * TQ],
                            kT[lo:lo + 64, mtl, j * P:(j + 1) * P],
                            q_sb[lo:lo + 64, mtl, :], start=True, stop=True)
                    p_t = ppool.tile([P, 2 * TQ], bf16, tag="p",
                                     name=f"p{h}_{pr}")
                    nc.scalar.activation(p_t[:], s_ps[:], AF.Exp, scale=SCALE)
                    p_tiles.append(p_t)
                o_ps = obps.tile([P, TQ], f32, tag="o", name=f"o{h}")
                for pr in range(4):
                    for jj in range(2):
                        j = 2 * pr + jj
                        nc.tensor.matmul(
                            o_ps[0:HD + 1, :], v_sb[:, j, h, :],
                            p_tiles[pr][:, jj * TQ:(jj + 1) * TQ],
                            start=(j == 0), stop=(j == NJ - 1))
                srow = sp.tile([1, TQ], bf16, tag="srow", name=f"sr{h}")
                nc.vector.reciprocal(srow[0:1, :], o_ps[HD:HD + 1, :])
                b_ps = obps.tile([P, TQ], f32, tag="o", name=f"b{h}")
                nc.tensor.matmul(b_ps[0:HD, :], onesr[:, 0:HD], srow[:],
                                 start=True, stop=True)
                b_sb = tmp_pool.tile([P, TQ], bf16, tag="tm", name=f"bs{h}")
                nc.scalar.copy(b_sb[0:HD, :], b_ps[0:HD, :])
                nc.vector.tensor_tensor(o_sb[lo:lo + 64, mtl, :],
                                        o_ps[0:HD, :], b_sb[0:HD, :],
                                        OP.mult)

            for mt in range(MT):
                k_proj_mt(mt)
                attn_head(2 * mt)
                attn_head(2 * mt + 1)

            # hoist the exp->sqrt act-table switch into Act idle time
            dummy = sp.tile([1, 1], f32, tag="dummy")
            nc.scalar.activation(dummy[:], eps_t[0:1, 0:1], AF.Sqrt)

            # ---- output projection + bias + residual -> xres ----
            xres = pp.tile([P, MT, TQ], bf16, tag="big_a", name="xres")
            for mt in range(MT):
                ps = mmps.tile([P, TQ], f32, tag="mm")
                for kt in range(MT):
                    nc.tensor.matmul(ps[:], wbig["wp"][:, mt, kt, :],
                                     o_sb[:, kt, :],
                                     start=(kt == 0), stop=(kt == MT - 1))
                nc.vector.scalar_tensor_tensor(
                    xres[:, mt, :], ps[:], vec["bp"][:, mt:mt + 1],
                    xqT[:, mt, :], OP.add, OP.add)

            def layernorm(src, nm, write_fn, gname=None, bname=None):
                sum_ps = sps.tile([1, TQ], f32, tag="s", name=f"lnsum_{nm}")
                sq_ps = sps.tile([1, TQ], f32, tag="s", name=f"lnsq_{nm}")
                for mt in range(MT):
                    sq = sqp.tile([P, TQ], bf16, tag="sq")
                    nc.vector.tensor_tensor(sq[:], src[:, mt, :], src[:, mt, :],
                                            OP.mult)
                    nc.tensor.matmul(sum_ps[:], onesc[:], src[:, mt, :],
                                     start=(mt == 0), stop=(mt == MT - 1))
                    nc.tensor.matmul(sq_ps[:], onesc[:], sq[:],
                                     start=(mt == 0), stop=(mt == MT - 1))
                mean = sp.tile([1, TQ], f32, tag="srow", name=f"mean_{nm}")
                nc.vector.tensor_scalar_mul(mean[:], sum_ps[:], 1.0 / DIM)
                msq = sp.tile([1, TQ], f32, tag="srow", name=f"msq_{nm}")
                nc.vector.tensor_tensor(msq[:], mean[:], mean[:], OP.mult)
                var = sp.tile([1, TQ], f32, tag="srow", name=f"var_{nm}")
                nc.vector.scalar_tensor_tensor(var[:], sq_ps[:], 1.0 / DIM,
                                               msq[:], OP.mult, OP.subtract)
                std = sp.tile([1, TQ], f32, tag="srow", name=f"std_{nm}")
                nc.scalar.activation(std[:], var[:], AF.Sqrt,
                                     bias=eps_t[0:1, 0:1])
                rstd = sp.tile([1, TQ], bf16, tag="srow", name=f"rstd_{nm}")
                nc.vector.reciprocal(rstd[:], std[:])
                nmr = sp.tile([1, TQ], bf16, tag="srow", name=f"nmr_{nm}")
                nc.vector.scalar_tensor_tensor(nmr[:], mean[:], -1.0, rstd[:],
                                               OP.mult, OP.mult)
                a_ps = obps.tile([P, TQ], f32, tag="o", name=f"arep_{nm}")
                nc.tensor.matmul(a_ps[:], onesr[:], rstd[:],
                                 start=True, stop=True)
                b_ps = obps.tile([P, TQ], f32, tag="o", name=f"brep_{nm}")
                nc.tensor.matmul(b_ps[:], onesr[:], nmr[:],
                                 start=True, stop=True)
                a_sb = sqp.tile([P, TQ], bf16, tag="sq", name=f"asb_{nm}")
                nc.scalar.copy(a_sb[:], a_ps[:])
                b_sb = sqp.tile([P, TQ], bf16, tag="sq", name=f"bsb_{nm}")
                nc.scalar.copy(b_sb[:], b_ps[:])
                for mt in range(MT):
                    dst = write_fn(mt)
                    eng = nc.vector if mt % 4 != 3 else nc.gpsimd
                    eng.tensor_tensor(dst, src[:, mt, :], a_sb[:], OP.mult)
                    eng.tensor_tensor(dst, dst, b_sb[:], OP.add)
                    if gname is not None:
                        eng.tensor_scalar(dst, dst, vec[gname][:, mt:mt + 1],
                                          vec[bname][:, mt:mt + 1],
                                          OP.mult, OP.add)

            # ---- LN1 -> x1 (normalized, affine folded into FFN + merge) ----
            x1 = pp.tile([P, MT, TQ], bf16, tag="x1")
            layernorm(xres, "l1", lambda mt: x1[:, mt, :])

            # ---- FFN1: relu(W1' x1 + bf1') -> hf ----
            hf = pp.tile([P, MT, TQ], bf16, tag="big_b", name="hf")
            for mt in range(MT):
                ps = mmps.tile([P, TQ], f32, tag="mm")
                for kt in range(MT):
                    nc.tensor.matmul(ps[:], wbig["w1"][:, mt, kt, :],
                                     x1[:, kt, :],
                                     start=(kt == 0), stop=(kt == MT - 1))
                nc.vector.tensor_scalar(hf[:, mt, :], ps[:],
                                        vec["bf1"][:, mt:mt + 1], 0.0,
                                        OP.add, OP.max)

            # ---- FFN2 + bf2 + residual x1_true -> yres ----
            # x1_true = x1*g1 + b1; yres = ffn2 + bf2 + x1_true
            yres = pp.tile([P, MT, TQ], bf16, tag="big_a", name="yres")
            for mt in range(MT):
                tm = tmp_pool.tile([P, TQ], bf16, tag="tm", name=f"tm{mt}")
                # SBUF-only op: fine on Pool, which is otherwise idle here
                nc.gpsimd.tensor_scalar(tm[:], x1[:, mt, :],
                                        vec["g1"][:, mt:mt + 1],
                                        vec["b1f2"][:, mt:mt + 1],
                                        OP.mult, OP.add)
                ps = mmps.tile([P, TQ], f32, tag="mm")
                for kt in range(MT):
                    nc.tensor.matmul(ps[:], wbig["w2"][:, mt, kt, :],
                                     hf[:, kt, :],
                                     start=(kt == 0), stop=(kt == MT - 1))
                nc.vector.tensor_tensor(yres[:, mt, :], tm[:], ps[:], OP.add)

            # ---- LN2 -> normalized out (host applies g2/b2), 4 DMA chunks ----
            o_out = pp.tile([P, MT, TQ], bf16, tag="o_out")
            layernorm(yres, "l2", lambda mt: o_out[:, mt, :],
                      gname="g2", bname="b2")
            for c in range(4):
                eng = nc.sync if c % 2 == 0 else nc.scalar
                eng.dma_start(out_d[:, 2 * c:2 * c + 2],
                              o_out[:, 2 * c:2 * c + 2, :])

    nc.compile()
    return nc


def _prep_core(inputs, b, qh, host):
    d = {
        "xqT": host["xqT"][b][qh],
        "xkT": host["xkT"][b],
        "xvT": host["xvT"][b],
        "mcol": host["mcol"][b],
    }
    d.update(host["shared"])
    return d


def _host_prep(inputs):
    import ml_dtypes
    bf = ml_dtypes.bfloat16

    def xt(x):
        # [n, d] -> [p, mt, n] bf16
        return np.ascontiguousarray(
            x.T.reshape(MT, P, x.shape[0]).transpose(1, 0, 2)).astype(bf)

    def wtiles(w):
        wt = w.T  # [k, m]
        return np.ascontiguousarray(
            wt.reshape(MT, P, MT, P).transpose(2, 1, 0, 3)).astype(bf)

    def wbig(w):
        # [p_k, mt_m, mt_k, p_m]: 16KB/partition contiguous, one DMA
        wt = w.T  # [k, m]
        return np.ascontiguousarray(
            wt.reshape(MT, P, MT, P).transpose(1, 2, 0, 3)).astype(bf)

    def vecp(v):
        return np.ascontiguousarray(v.reshape(MT, P).T.astype(np.float32))

    mask = inputs["mask"].astype(np.float32)          # [B, NKV] of 0/1
    xv_masked = inputs["x_v"] * mask[:, :, None]

    g1 = inputs["g_ln1"].astype(np.float32)
    b1 = inputs["b_ln1"].astype(np.float32)
    w1_folded = inputs["W1"].astype(np.float32) * g1[None, :]
    bf1_folded = (inputs["bf1"].astype(np.float32)
                  + inputs["W1"].astype(np.float32) @ b1)

    host = {
        "xqT": [[xt(inputs["x_q"][b, qh * TQ:(qh + 1) * TQ, :])
                 for qh in range(2)] for b in range(B)],
        "xkT": [xt(inputs["x_k"][b]) for b in range(B)],
        "xvT": [xt(xv_masked[b]) for b in range(B)],
        "mcol": [np.ascontiguousarray(mask[b].reshape(NJ, P).T).astype(bf)
                 for b in range(B)],
        "shared": {
            "onesc": np.ones((P, 1), bf),
            "onesr": np.ones((1, P), bf),
            "wv_r": np.ascontiguousarray(
                inputs["Wv"].T.reshape(MT, P, DIM).transpose(1, 0, 2)).astype(bf),
            "wk": wbig(inputs["Wk"]),
            "wq": wtiles(inputs["Wq"]),
            "wp": wbig(inputs["Wp"]),
            "w1": wbig(w1_folded),
            "w2": wbig(inputs["W2"]),
            "bp": vecp(inputs["bp"]),
            "bf1": vecp(bf1_folded),
            "g1": vecp(g1),
            "b1f2": vecp(b1 + inputs["bf2"].astype(np.float32)),
            "g2": vecp(inputs["g_ln2"].astype(np.float32)),
            "b2": vecp(inputs["b_ln2"].astype(np.float32)),
        },
    }
    return host


def get_nc():
    if "nc" not in _CACHE:
        _CACHE["nc"] = _build()
    return _CACHE["nc"]


def _unshard(host, results):
    out = np.empty((B, NQ, DIM), np.float32)
    for c in range(8):
        b, qh = c // 2, c % 2
        oc = np.asarray(results[c]["out"]).astype(np.float32)  # [p, mt, q]
        out[b, qh * TQ:(qh + 1) * TQ, :] = (
            oc.transpose(2, 1, 0).reshape(TQ, DIM))
    return out


def kernel(**inputs):
    from concourse.bass_utils import run_bass_kernel_spmd
    inputs = {k: np.asarray(v) for k, v in inputs.items()}
    nc = get_nc()
    host = _host_prep(inputs)
    in_maps = []
    for c in range(8):
        in_maps.append(_prep_core(inputs, c // 2, c % 2, host))
    res = run_bass_kernel_spmd(nc, in_maps, list(range(8)))
    return _unshard(host, res.results)
